# revision 34
# baseline (speedup 1.0000x reference)
"""Trainium2 Bass kernel for ragged phonology-embedding mean + position add.

Reference semantics (per (b, s)):
    out[b, s, :] = mean_{g < len[b,s]} table[tok[b,s,g], :] + pos[s, :]

v2 strategy (data-parallel over B across 8 cores). Per core there are 16
output tiles of 128 rows; each tile's masked ragged mean is a block-sparse
matmul out_t = W_t @ table. Two fused compute paths per tile, accumulated
in one PSUM:

  - HOT vocab rows (top HCH*128 rows by #tiles-using-them, per core): kept
    resident in SBUF as HCH K-chunks; every tile matmuls all hot chunks
    with its host-built weight block. Replaces the v1 full-table one-hot
    path at a fraction of the PE/DMA cost (no 4 MB tablek load).
  - COLD rows: per tile-PAIR dma_gather of the deduped cold tokens ordered
    [A-only, shared, B-only] so boundary chunks serve both tiles. Q7
    desc-gen (~9.5 ns/idx) runs serially; calls sized/split so the first
    data lands early and the tail call is small.

Schedule notes (HW-calibrated): the first dma_gather cannot execute
before ~17.5us (preamble + async Q7 microcode load - measured floor on a
minimal kernel), so the schedule front-loads PE p-state warmup and
hot-chunk matmuls, paces const DMAs to keep the startup HBM window
quiet, and windows the gather calls over the global chunk axis (small
first calls prime PSUM turnover, small last call shrinks the tail).
Drains are split across idle engines: ACT copies PSUM->bf16 (releasing
the PSUM bank early), DVE adds the position table in all-bf16 4x mode.
Outputs are written bf16 and upcast on host.
"""

import os
import numpy as np
import ml_dtypes

import concourse.bass as bass
import concourse.bacc as bacc
import concourse.mybir as mybir
import concourse.tile as tile
from concourse.bass_utils import run_bass_kernel_spmd

B, S, G = 128, 128, 8
VOCAB, D = 2048, 1024
NCORES = 8
BPC = B // NCORES          # batches per core
R = BPC * S                # rows (b,s pairs) per core
P = 128
NT = R // P                # output tiles per core
MAXCH = 8                  # dma_gather HW cap: 1024 indices per call

# engine-cost model (ns), calibrated on HW traces (2026-08 session)
MM_NS = 262.0              # warm bf16 matmul [128x128]@[128x512]
# first-gather floor is structural: ~6.5us preamble + ~10us async Q7
# microcode load after LIBRARY_RELOAD (measured on a minimal kernel)
Q7_START_NS = 16_800.0
Q7_IDX_NS = 7.2            # serial Q7 desc-gen per gathered index (warm)
Q7_CALL_NS = 620.0         # per-call fixed overhead
READY_NS = 1_800.0         # SDMA drain from call end to data-in-SBUF
PE_START_NS = 10_300.0     # end of p-state warmup / first real matmul
TAIL_NS = 5_500.0          # drain + out DMA + NEFF epilogue


def _cdiv(a, b):
    return -(-a // b)


def _env(name, default):
    return int(os.environ.get(name, default))


def _prepare(phon_tokens, group_len_raw):
    toks = np.asarray(phon_tokens).astype(np.int64).reshape(B, S, G)
    lens = (np.asarray(group_len_raw).astype(np.int64) + 1).reshape(B, S)
    assert lens.min() >= 1 and lens.max() <= G
    assert toks.min() >= 0 and toks.max() < VOCAB

    toks_c = toks.reshape(NCORES, NT, P, G)
    lens_c = lens.reshape(NCORES, NT, P)

    # per (core, tile): dedup tokens; weights carry count/len.
    uniqs = {}
    wmats = {}
    cnt = np.zeros((NCORES, VOCAB), np.int32)
    for c in range(NCORES):
        for t in range(NT):
            tl = toks_c[c, t]
            ll = lens_c[c, t]
            valid = np.arange(G)[None, :] < ll[:, None]
            flat = tl[valid]
            pair = np.repeat(np.arange(P), ll)
            uniq, inv = np.unique(flat, return_inverse=True)
            wm = np.zeros((uniq.size, P), np.float32)
            np.add.at(wm, (inv, pair), 1.0 / ll[pair])
            uniqs[c, t] = uniq
            wmats[c, t] = wm
            cnt[c, uniq] += 1

    def _make_plan(hch):
        # hot rows per core: top hch*128 by tile-use count
        hots = {}
        for c in range(NCORES):
            hot = np.sort(np.argsort(-cnt[c], kind="stable")[: hch * P])
            hots[c] = hot
        cold = {}
        for c in range(NCORES):
            for t in range(NT):
                cold[c, t] = np.setdiff1d(uniqs[c, t], hots[c],
                                          assume_unique=True)

        # pair groups, ordered by descending worst-core union size so the
        # tail call is small
        pairs = [(2 * i, 2 * i + 1) for i in range(NT // 2)]
        usize = []
        for (ta, tb) in pairs:
            usize.append(max(
                np.union1d(cold[c, ta], cold[c, tb]).size
                for c in range(NCORES)
            ))
        order = np.argsort(-np.asarray(usize), kind="stable")
        pairs = [pairs[i] for i in order]

        # per group, per core: packed order [A-only, shared, B-only]
        ginfo = []
        goff = 0
        for (ta, tb) in pairs:
            pc = {}
            nA = np.zeros(NCORES, int)
            nB0 = np.zeros(NCORES, int)
            nU = np.zeros(NCORES, int)
            for c in range(NCORES):
                a, b = cold[c, ta], cold[c, tb]
                sh = np.intersect1d(a, b, assume_unique=True)
                aonly = np.setdiff1d(a, sh, assume_unique=True)
                bonly = np.setdiff1d(b, sh, assume_unique=True)
                pc[c] = np.concatenate([aonly, sh, bonly])
                nA[c] = aonly.size + sh.size
                nB0[c] = aonly.size
                nU[c] = pc[c].size
            nch = int(_cdiv(nU.max(), P))
            ginfo.append(dict(tiles=(ta, tb), packed=pc, nA=nA, nB0=nB0,
                              nU=nU, nch=nch, off=goff))
            goff += nch
        total_chunks = goff

        # map global chunk -> (group, local chunk)
        chunk_owner = []
        for k, g in enumerate(ginfo):
            chunk_owner.extend((k, j) for j in range(g["nch"]))

        # call windows over the global chunk axis; first calls small so the
        # first drains (and PSUM turnover) start as early as possible, last
        # call small for the tail
        CSZ = _env("CSZ", 6)
        sizes = [2, 3]
        while sum(sizes) < total_chunks - 2:
            sizes.append(min(CSZ, total_chunks - 2 - sum(sizes)))
        sizes.append(2)
        ncalls = len(sizes)

        calls = []
        entry_off = 0
        c0 = 0
        for nch in sizes:
            entries = []
            for j in range(nch):
                if c0 + j >= len(chunk_owner):
                    break
                k, jl = chunk_owner[c0 + j]
                g = ginfo[k]
                ta, tb = g["tiles"]
                jlo, jhi = jl * P, (jl + 1) * P
                if any(jlo < g["nA"][c] for c in range(NCORES)):
                    entries.append((j, ta))
                if any(jhi > g["nB0"][c] and jlo < g["nU"][c]
                       for c in range(NCORES)):
                    entries.append((j, tb))
            calls.append(dict(nch=nch, num_idxs=nch * P, idx_base=c0,
                              entry_base=entry_off, entries=entries))
            entry_off += len(entries)
            c0 += nch

        # global first/last gather entry per tile (for PSUM stop + drain)
        glast = {}
        for ci, call in enumerate(calls):
            for e, (j, t) in enumerate(call["entries"]):
                glast[t] = (ci, e)
        for ci, call in enumerate(calls):
            call["last"] = {t: e for t, (c_, e) in glast.items() if c_ == ci}

        return dict(hch=hch, hots=hots, cold=cold, ginfo=ginfo, calls=calls,
                    chunk_owner=chunk_owner, total_chunks=sum(sizes),
                    total_entries=entry_off)

    def _sim_cost(plan):
        hch = plan["hch"]
        calls = plan["calls"]
        q7 = Q7_START_NS
        ready = []
        for call in calls:
            q7 += Q7_CALL_NS + call["num_idxs"] * Q7_IDX_NS
            ready.append(q7 + READY_NS)
        pe = PE_START_NS
        hot_dur = hch * 2 * MM_NS
        # hot fillers must follow drain (group) order or PSUM reuse deadlocks
        tile_order = [t for g in plan["ginfo"] for t in g["tiles"]]
        hot_done = {t: False for t in tile_order}
        ndone = 0
        ndrained = 0
        nexth = 0
        for ci, call in enumerate(calls):
            ctiles = sorted({t for _, t in call["entries"]})
            while pe < ready[ci] and nexth < NT:
                t = tile_order[nexth]
                if hot_done[t]:
                    nexth += 1
                    continue
                if ndone - ndrained >= 4:
                    break
                hot_done[t] = True
                ndone += 1
                pe += hot_dur
            for t in ctiles:
                if not hot_done[t]:
                    hot_done[t] = True
                    ndone += 1
                    pe += hot_dur
            pe = max(pe, ready[ci]) + len(call["entries"]) * 2 * MM_NS
            ndrained += len(call["last"])
        return max(pe, q7 + READY_NS) + TAIL_NS

    hch_env = _env("HCH", -1)
    opts = [hch_env] if hch_env > 0 else [1, 2, 3, 4, 5, 6]
    plans = [(_sim_cost(p), p) for p in (_make_plan(h) for h in opts)]
    if os.environ.get("PLAN_DEBUG"):
        for est, p in plans:
            tot = sum(c["num_idxs"] for c in p["calls"])
            ent = p["total_entries"]
            print(f"[plan] hch={p['hch']} est={est/1000:.1f}us idx={tot} "
                  f"entries={ent} calls={len(p['calls'])}")
    plans.sort(key=lambda x: x[0])
    est, plan = plans[0]
    plan["est_ns"] = est

    # ---- build host arrays ----
    wdt = ml_dtypes.bfloat16
    hch = plan["hch"]
    calls = plan["calls"]
    ginfo = plan["ginfo"]
    total_chunks = max(plan["total_chunks"], 1)
    total_entries = max(plan["total_entries"], 1)

    idx_all = np.zeros((NCORES, total_chunks * P), np.int64)
    w_all = np.zeros((NCORES, total_entries, P, P), np.float32)
    chot_all = np.zeros((NCORES, NT, hch * P, P), np.float32)
    for c in range(NCORES):
        hot = plan["hots"][c]
        hpos = {v: i for i, v in enumerate(hot)}
        for t in range(NT):
            uniq = uniqs[c, t]
            wm = wmats[c, t]
            inhot = np.isin(uniq, hot, assume_unique=True)
            rows = np.array([hpos[v] for v in uniq[inhot]], int)
            if rows.size:
                chot_all[c, t, rows, :] = wm[inhot]
        for g in ginfo:
            pc = g["packed"][c]
            idx_all[c, g["off"] * P:g["off"] * P + pc.size] = pc
        for call in calls:
            for e, (j, tt) in enumerate(call["entries"]):
                k, jl = plan["chunk_owner"][call["idx_base"] + j]
                g = ginfo[k]
                ta, tb = g["tiles"]
                pc = g["packed"][c]
                slo, shi = jl * P, min((jl + 1) * P, pc.size)
                if slo >= shi:
                    continue
                sub = pc[slo:shi]
                gpos = np.arange(slo, shi)
                side = gpos < g["nA"][c] if tt == ta else gpos >= g["nB0"][c]
                if not side.any():
                    continue
                uu = uniqs[c, tt]
                wm = wmats[c, tt]
                rows = np.searchsorted(uu, sub[side])
                w_all[c, call["entry_base"] + e, np.nonzero(side)[0]] = (
                    wm[rows]
                )

    idx_maps, w_maps, chot_maps, hot_maps = [], [], [], []
    for c in range(NCORES):
        idxw = np.tile(idx_all[c].reshape(-1, 16).T, (8, 1)).astype(np.int16)
        idx_maps.append(np.ascontiguousarray(idxw))
        wf = w_all[c].transpose(1, 0, 2).reshape(P, -1).astype(wdt)
        w_maps.append(np.ascontiguousarray(wf))
        cf = (
            chot_all[c]
            .reshape(NT, hch, P, P)
            .transpose(2, 0, 1, 3)
            .reshape(P, -1)
            .astype(wdt)
        )
        chot_maps.append(np.ascontiguousarray(cf))
        hot_maps.append(plan["hots"][c])

    return plan, idx_maps, w_maps, chot_maps, hot_maps


def _build_nc(plan):
    mdt = mybir.dt.bfloat16
    f32 = mybir.dt.float32
    hch = plan["hch"]
    calls = plan["calls"]
    ginfo = plan["ginfo"]
    total_chunks = max(plan["total_chunks"], 1)
    total_entries = max(plan["total_entries"], 1)
    max_entries = max((len(c["entries"]) for c in calls), default=1)
    NWARM = _env("NWARM", 4)

    nc = bacc.Bacc("TRN2", target_bir_lowering=False, debug=False)

    table_d = nc.dram_tensor("table", [VOCAB, D], mdt, kind="ExternalInput")
    hot_d = nc.dram_tensor("hotrows", [P, hch * D], mdt, kind="ExternalInput")
    chot_d = nc.dram_tensor("chot", [P, NT * hch * P], mdt,
                            kind="ExternalInput")
    pos_d = nc.dram_tensor("pos", [P, D], mdt, kind="ExternalInput")
    idx_d = nc.dram_tensor("idxs", [P, total_chunks * 8], mybir.dt.int16,
                           kind="ExternalInput")
    w_d = nc.dram_tensor("wmat", [P, total_entries * P], mdt,
                         kind="ExternalInput")
    out_d = nc.dram_tensor("out", [R, D], mdt, kind="ExternalOutput")

    # build-time schedule: same policy as the cost sim in _prepare
    q7 = Q7_START_NS
    ready = []
    for call in calls:
        q7 += Q7_CALL_NS + call["num_idxs"] * Q7_IDX_NS
        ready.append(q7 + READY_NS)
    hot_dur = hch * 2 * MM_NS
    sched = []               # ("h", t) | ("c", ci)
    tile_order = [t for g in ginfo for t in g["tiles"]]
    hot_done = {t: False for t in tile_order}
    ndone = 0
    ndrained = 0
    pe = PE_START_NS
    nexth = 0
    for ci, call in enumerate(calls):
        assert call["nch"] <= MAXCH
        ctiles = sorted({t for _, t in call["entries"]})
        while pe < ready[ci] and nexth < NT:
            t = tile_order[nexth]
            if hot_done[t]:
                nexth += 1
                continue
            if ndone - ndrained >= 4:
                break
            hot_done[t] = True
            ndone += 1
            sched.append(("h", t))
            pe += hot_dur
        for t in ctiles:
            if not hot_done[t]:
                hot_done[t] = True
                ndone += 1
                sched.append(("h", t))
                pe += hot_dur
        sched.append(("c", ci))
        pe = max(pe, ready[ci]) + len(call["entries"]) * 2 * MM_NS
        ndrained += len(call["last"])
    for t in tile_order:
        if not hot_done[t]:
            sched.append(("h", t))

    # num_idxs registers hoisted before the tile scope so the MOVEs run
    # during the preamble, not on the critical Pool-sequencer path
    nregs = {}
    for call in calls:
        n = call["num_idxs"]
        if n not in nregs:
            nregs[n] = nc.gpsimd.to_reg(n)

    with tile.TileContext(nc) as tc:
        with (
            tc.tile_pool(name="const", bufs=1) as cpool,
            tc.tile_pool(name="gather", bufs=8) as gpool,
            tc.tile_pool(name="wpool", bufs=4) as wpool,
            tc.tile_pool(name="osb", bufs=6) as opool,
            tc.tile_pool(name="psum", bufs=4, space=bass.MemorySpace.PSUM) as ppool,
        ):
            # keep the early HBM burst minimal: a saturated HBM window at
            # kernel start starves the Pool sequencer and delays the first
            # gather by ~10us. Load idx + hot rows + the first few chot
            # slices now; pace the rest between gather calls.
            idx_sb = cpool.tile([P, total_chunks * 8], mybir.dt.int16)
            nc.sync.dma_start(idx_sb[:], idx_d[:])
            hot_sb = cpool.tile([P, hch * D], mdt)
            nc.sync.dma_start(hot_sb[:], hot_d[:])
            chot_sb = cpool.tile([P, NT * hch * P], mdt)

            def load_chot(t):
                lo, hi = t * hch * P, (t + 1) * hch * P
                nc.sync.dma_start(chot_sb[:, lo:hi], chot_d[:, lo:hi])

            chot_loaded = set()
            for t in tile_order[:4]:
                load_chot(t)
                chot_loaded.add(t)
            pos_sb = cpool.tile([P, D], mdt)

            # PE p-state warmup on a zeroed dummy block
            wz = cpool.tile([P, P + 512], mdt)
            nc.scalar.memzero(wz[:])
            if NWARM:
                pwarm = ppool.tile([P, 512], f32, tag="ps")
                for i in range(NWARM):
                    nc.tensor.matmul(
                        pwarm[:],
                        lhsT=wz[:, :P],
                        rhs=wz[:, P:P + 512],
                        start=(i == 0),
                        stop=(i == NWARM - 1),
                    )

            def prefetch_chot(n):
                for t in tile_order:
                    if n <= 0:
                        break
                    if t not in chot_loaded:
                        load_chot(t)
                        chot_loaded.add(t)
                        n -= 1

            psums = {}
            ncalls_done = 0
            for kind, item in sched:
                if kind == "h":
                    t = item
                    if t not in chot_loaded:
                        load_chot(t)
                        chot_loaded.add(t)
                    psums[t] = ppool.tile([P, D], f32, tag="ps", name="ps")
                    for k in range(hch):
                        base = (t * hch + k) * P
                        for h in range(0, D, 512):
                            nc.tensor.matmul(
                                psums[t][:, h:h + 512],
                                lhsT=chot_sb[:, base:base + P],
                                rhs=hot_sb[:, k * D + h:k * D + h + 512],
                                start=(k == 0),
                                stop=False,
                            )
                else:
                    call = calls[item]
                    nch = call["nch"]
                    b0 = call["idx_base"]
                    prefetch_chot(3)
                    if ncalls_done == 1:
                        nc.sync.dma_start(pos_sb[:], pos_d[:])
                    ncalls_done += 1
                    gt = gpool.tile([P, MAXCH, D], mdt, tag="gt")
                    nc.gpsimd.dma_gather(
                        gt[:, :nch, :],
                        table_d[:],
                        idx_sb[:, b0 * 8:(b0 + nch) * 8],
                        num_idxs=call["num_idxs"],
                        num_idxs_reg=nregs[call["num_idxs"]],
                        elem_size=D,
                    )
                    ne = len(call["entries"])
                    wt = wpool.tile([P, max_entries * P], mdt, tag="wt")
                    eb = call["entry_base"]
                    nc.sync.dma_start(
                        wt[:, :ne * P], w_d[:, eb * P:(eb + ne) * P]
                    )
                    for e, (j, t) in enumerate(call["entries"]):
                        last = call["last"].get(t, -1) == e
                        for h in range(0, D, 512):
                            nc.tensor.matmul(
                                psums[t][:, h:h + 512],
                                lhsT=wt[:, e * P:(e + 1) * P],
                                rhs=gt[:, j, h:h + 512],
                                start=False,
                                stop=last,
                            )
                        if last:
                            o1 = opool.tile([P, D], mdt, tag="o1")
                            nc.scalar.copy(o1[:], psums[t][:])
                            o2 = opool.tile([P, D], mdt, tag="o2")
                            nc.vector.tensor_tensor(
                                o2[:], o1[:], pos_sb[:],
                                op=mybir.AluOpType.add,
                            )
                            nc.sync.dma_start(
                                out_d[t * P:(t + 1) * P, :], o2[:]
                            )
    nc.compile()
    return nc


def run(inputs, trace=False, tmpdir=None):
    """Returns (out [B,S,D] f32, BassKernelResults)."""
    plan, idx_maps, w_maps, chot_maps, hot_maps = _prepare(
        inputs["phon_tokens"], inputs["group_len_raw"]
    )
    wdt = ml_dtypes.bfloat16
    hch = plan["hch"]
    table_np = np.ascontiguousarray(
        np.asarray(inputs["phon_emb_table"]).astype(wdt)
    )
    pos_np = np.ascontiguousarray(
        np.asarray(inputs["pos_emb_table"]).astype(wdt)
    )

    nc = _build_nc(plan)
    in_maps = []
    for c in range(NCORES):
        hot_rows = table_np[hot_maps[c]]          # [hch*128, D]
        hot_np = np.ascontiguousarray(
            hot_rows.reshape(hch, P, D).transpose(1, 0, 2).reshape(P, hch * D)
        )
        in_maps.append({
            "table": table_np, "hotrows": hot_np, "chot": chot_maps[c],
            "pos": pos_np, "idxs": idx_maps[c], "wmat": w_maps[c],
        })
    res = run_bass_kernel_spmd(
        nc, in_maps, core_ids=list(range(NCORES)), trace=trace, tmpdir=tmpdir
    )
    out = np.empty((B, S, D), np.float32)
    for c in range(NCORES):
        out[c * BPC:(c + 1) * BPC] = (
            res.results[c]["out"].astype(np.float32).reshape(BPC, S, D)
        )
    return out, res


def kernel(**inputs) -> np.ndarray:
    out, _ = run(inputs, trace=False)
    return out


# revision 35
# speedup vs baseline: 1.0451x; 1.0451x over previous
"""Trainium2 Bass kernel for ragged phonology-embedding mean + position add.

Reference semantics (per (b, s)):
    out[b, s, :] = mean_{g < len[b,s]} table[tok[b,s,g], :] + pos[s, :]

v2 strategy (data-parallel over B across 8 cores). Per core there are 16
output tiles of 128 rows; each tile's masked ragged mean is a block-sparse
matmul out_t = W_t @ table. Two fused compute paths per tile, accumulated
in one PSUM:

  - HOT vocab rows (top HCH*128 rows by #tiles-using-them, per core): kept
    resident in SBUF as HCH K-chunks; every tile matmuls all hot chunks
    with its host-built weight block. Replaces the v1 full-table one-hot
    path at a fraction of the PE/DMA cost (no 4 MB tablek load).
  - COLD rows: per tile-PAIR dma_gather of the deduped cold tokens ordered
    [A-only, shared, B-only] so boundary chunks serve both tiles. Q7
    desc-gen (~9.5 ns/idx) runs serially; calls sized/split so the first
    data lands early and the tail call is small.

Schedule notes (HW-calibrated): the first dma_gather cannot execute
before ~17.5us (preamble + async Q7 microcode load - measured floor on a
minimal kernel), so the schedule front-loads PE p-state warmup and
hot-chunk matmuls, paces const DMAs to keep the startup HBM window
quiet, and windows the gather calls over the global chunk axis (small
first calls prime PSUM turnover, small last call shrinks the tail).
Drains are split across idle engines: ACT copies PSUM->bf16 (releasing
the PSUM bank early), DVE adds the position table in all-bf16 4x mode.
Outputs are written bf16 and upcast on host.
"""

import os
import numpy as np
import ml_dtypes

import concourse.bass as bass
import concourse.bacc as bacc
import concourse.mybir as mybir
import concourse.tile as tile
from concourse.bass_utils import run_bass_kernel_spmd

B, S, G = 128, 128, 8
VOCAB, D = 2048, 1024
NCORES = 8
BPC = B // NCORES          # batches per core
R = BPC * S                # rows (b,s pairs) per core
P = 128
NT = R // P                # output tiles per core
MAXCH = 8                  # dma_gather HW cap: 1024 indices per call

# engine-cost model (ns), calibrated on HW traces (2026-08 session)
MM_NS = 262.0              # warm bf16 matmul [128x128]@[128x512]
# first-gather floor is structural: ~6.5us preamble + ~10us async Q7
# microcode load after LIBRARY_RELOAD (measured on a minimal kernel)
Q7_START_NS = 16_800.0
Q7_IDX_NS = 7.2            # serial Q7 desc-gen per gathered index (warm)
Q7_CALL_NS = 620.0         # per-call fixed overhead
READY_NS = 1_800.0         # SDMA drain from call end to data-in-SBUF
PE_START_NS = 10_300.0     # end of p-state warmup / first real matmul
TAIL_NS = 5_500.0          # drain + out DMA + NEFF epilogue


def _cdiv(a, b):
    return -(-a // b)


def _env(name, default):
    return int(os.environ.get(name, default))


def _prepare(phon_tokens, group_len_raw):
    toks = np.asarray(phon_tokens).astype(np.int64).reshape(B, S, G)
    lens = (np.asarray(group_len_raw).astype(np.int64) + 1).reshape(B, S)
    assert lens.min() >= 1 and lens.max() <= G
    assert toks.min() >= 0 and toks.max() < VOCAB

    toks_c = toks.reshape(NCORES, NT, P, G)
    lens_c = lens.reshape(NCORES, NT, P)

    # per (core, tile): dedup tokens; weights carry count/len.
    uniqs = {}
    wmats = {}
    cnt = np.zeros((NCORES, VOCAB), np.int32)
    for c in range(NCORES):
        for t in range(NT):
            tl = toks_c[c, t]
            ll = lens_c[c, t]
            valid = np.arange(G)[None, :] < ll[:, None]
            flat = tl[valid]
            pair = np.repeat(np.arange(P), ll)
            uniq, inv = np.unique(flat, return_inverse=True)
            wm = np.zeros((uniq.size, P), np.float32)
            np.add.at(wm, (inv, pair), 1.0 / ll[pair])
            uniqs[c, t] = uniq
            wmats[c, t] = wm
            cnt[c, uniq] += 1

    def _make_plan(hch):
        # hot rows per core: top hch*128 by tile-use count
        hots = {}
        for c in range(NCORES):
            hot = np.sort(np.argsort(-cnt[c], kind="stable")[: hch * P])
            hots[c] = hot
        cold = {}
        for c in range(NCORES):
            for t in range(NT):
                cold[c, t] = np.setdiff1d(uniqs[c, t], hots[c],
                                          assume_unique=True)

        # pair groups, ordered by descending worst-core union size so the
        # tail call is small
        pairs = [(2 * i, 2 * i + 1) for i in range(NT // 2)]
        usize = []
        for (ta, tb) in pairs:
            usize.append(max(
                np.union1d(cold[c, ta], cold[c, tb]).size
                for c in range(NCORES)
            ))
        order = np.argsort(-np.asarray(usize), kind="stable")
        pairs = [pairs[i] for i in order]

        # per group, per core: packed order [A-only, shared, B-only]
        ginfo = []
        goff = 0
        for (ta, tb) in pairs:
            pc = {}
            nA = np.zeros(NCORES, int)
            nB0 = np.zeros(NCORES, int)
            nU = np.zeros(NCORES, int)
            for c in range(NCORES):
                a, b = cold[c, ta], cold[c, tb]
                sh = np.intersect1d(a, b, assume_unique=True)
                aonly = np.setdiff1d(a, sh, assume_unique=True)
                bonly = np.setdiff1d(b, sh, assume_unique=True)
                pc[c] = np.concatenate([aonly, sh, bonly])
                nA[c] = aonly.size + sh.size
                nB0[c] = aonly.size
                nU[c] = pc[c].size
            nch = int(_cdiv(nU.max(), P))
            ginfo.append(dict(tiles=(ta, tb), packed=pc, nA=nA, nB0=nB0,
                              nU=nU, nch=nch, off=goff))
            goff += nch
        total_chunks = goff

        # map global chunk -> (group, local chunk)
        chunk_owner = []
        for k, g in enumerate(ginfo):
            chunk_owner.extend((k, j) for j in range(g["nch"]))

        # call windows over the global chunk axis; first calls small so the
        # first drains (and PSUM turnover) start as early as possible, last
        # call small for the tail
        CSZ = _env("CSZ", 6)
        sizes = [2, 3]
        while sum(sizes) < total_chunks - 2:
            sizes.append(min(CSZ, total_chunks - 2 - sum(sizes)))
        sizes.append(2)
        ncalls = len(sizes)

        calls = []
        entry_off = 0
        c0 = 0
        for nch in sizes:
            entries = []
            for j in range(nch):
                if c0 + j >= len(chunk_owner):
                    break
                k, jl = chunk_owner[c0 + j]
                g = ginfo[k]
                ta, tb = g["tiles"]
                jlo, jhi = jl * P, (jl + 1) * P
                if any(jlo < g["nA"][c] for c in range(NCORES)):
                    entries.append((j, ta))
                if any(jhi > g["nB0"][c] and jlo < g["nU"][c]
                       for c in range(NCORES)):
                    entries.append((j, tb))
            calls.append(dict(nch=nch, num_idxs=nch * P, idx_base=c0,
                              entry_base=entry_off, entries=entries))
            entry_off += len(entries)
            c0 += nch

        # global first/last gather entry per tile (for PSUM stop + drain)
        glast = {}
        for ci, call in enumerate(calls):
            for e, (j, t) in enumerate(call["entries"]):
                glast[t] = (ci, e)
        for ci, call in enumerate(calls):
            call["last"] = {t: e for t, (c_, e) in glast.items() if c_ == ci}

        return dict(hch=hch, hots=hots, cold=cold, ginfo=ginfo, calls=calls,
                    chunk_owner=chunk_owner, total_chunks=sum(sizes),
                    total_entries=entry_off)

    def _sim_cost(plan):
        hch = plan["hch"]
        calls = plan["calls"]
        q7 = Q7_START_NS
        ready = []
        for call in calls:
            q7 += Q7_CALL_NS + call["num_idxs"] * Q7_IDX_NS
            ready.append(q7 + READY_NS)
        pe = PE_START_NS
        hot_dur = hch * 2 * MM_NS
        # hot fillers must follow drain (group) order or PSUM reuse deadlocks
        tile_order = [t for g in plan["ginfo"] for t in g["tiles"]]
        hot_done = {t: False for t in tile_order}
        ndone = 0
        ndrained = 0
        nexth = 0
        for ci, call in enumerate(calls):
            ctiles = sorted({t for _, t in call["entries"]})
            while pe < ready[ci] and nexth < NT:
                t = tile_order[nexth]
                if hot_done[t]:
                    nexth += 1
                    continue
                if ndone - ndrained >= 4:
                    break
                hot_done[t] = True
                ndone += 1
                pe += hot_dur
            for t in ctiles:
                if not hot_done[t]:
                    hot_done[t] = True
                    ndone += 1
                    pe += hot_dur
            pe = max(pe, ready[ci]) + len(call["entries"]) * 2 * MM_NS
            ndrained += len(call["last"])
        return max(pe, q7 + READY_NS) + TAIL_NS

    hch_env = _env("HCH", -1)
    opts = [hch_env] if hch_env > 0 else [4]
    plans = [(_sim_cost(p), p) for p in (_make_plan(h) for h in opts)]
    if os.environ.get("PLAN_DEBUG"):
        for est, p in plans:
            tot = sum(c["num_idxs"] for c in p["calls"])
            ent = p["total_entries"]
            print(f"[plan] hch={p['hch']} est={est/1000:.1f}us idx={tot} "
                  f"entries={ent} calls={len(p['calls'])}")
    plans.sort(key=lambda x: x[0])
    est, plan = plans[0]
    plan["est_ns"] = est

    # ---- build host arrays ----
    wdt = ml_dtypes.bfloat16
    hch = plan["hch"]
    calls = plan["calls"]
    ginfo = plan["ginfo"]
    total_chunks = max(plan["total_chunks"], 1)
    total_entries = max(plan["total_entries"], 1)

    idx_all = np.zeros((NCORES, total_chunks * P), np.int64)
    w_all = np.zeros((NCORES, total_entries, P, P), np.float32)
    chot_all = np.zeros((NCORES, NT, hch * P, P), np.float32)
    for c in range(NCORES):
        hot = plan["hots"][c]
        hpos = {v: i for i, v in enumerate(hot)}
        for t in range(NT):
            uniq = uniqs[c, t]
            wm = wmats[c, t]
            inhot = np.isin(uniq, hot, assume_unique=True)
            rows = np.array([hpos[v] for v in uniq[inhot]], int)
            if rows.size:
                chot_all[c, t, rows, :] = wm[inhot]
        for g in ginfo:
            pc = g["packed"][c]
            idx_all[c, g["off"] * P:g["off"] * P + pc.size] = pc
        for call in calls:
            for e, (j, tt) in enumerate(call["entries"]):
                k, jl = plan["chunk_owner"][call["idx_base"] + j]
                g = ginfo[k]
                ta, tb = g["tiles"]
                pc = g["packed"][c]
                slo, shi = jl * P, min((jl + 1) * P, pc.size)
                if slo >= shi:
                    continue
                sub = pc[slo:shi]
                gpos = np.arange(slo, shi)
                side = gpos < g["nA"][c] if tt == ta else gpos >= g["nB0"][c]
                if not side.any():
                    continue
                uu = uniqs[c, tt]
                wm = wmats[c, tt]
                rows = np.searchsorted(uu, sub[side])
                w_all[c, call["entry_base"] + e, np.nonzero(side)[0]] = (
                    wm[rows]
                )

    idx_maps, w_maps, chot_maps, hot_maps = [], [], [], []
    for c in range(NCORES):
        idxw = np.tile(idx_all[c].reshape(-1, 16).T, (8, 1)).astype(np.int16)
        idx_maps.append(np.ascontiguousarray(idxw))
        wf = w_all[c].transpose(1, 0, 2).reshape(P, -1).astype(wdt)
        w_maps.append(np.ascontiguousarray(wf))
        cf = (
            chot_all[c]
            .reshape(NT, hch, P, P)
            .transpose(2, 0, 1, 3)
            .reshape(P, -1)
            .astype(wdt)
        )
        chot_maps.append(np.ascontiguousarray(cf))
        hot_maps.append(plan["hots"][c])

    return plan, idx_maps, w_maps, chot_maps, hot_maps


def _build_nc(plan):
    mdt = mybir.dt.bfloat16
    f32 = mybir.dt.float32
    hch = plan["hch"]
    calls = plan["calls"]
    ginfo = plan["ginfo"]
    total_chunks = max(plan["total_chunks"], 1)
    total_entries = max(plan["total_entries"], 1)
    max_entries = max((len(c["entries"]) for c in calls), default=1)
    NWARM = _env("NWARM", 4)

    nc = bacc.Bacc("TRN2", target_bir_lowering=False, debug=False)

    table_d = nc.dram_tensor("table", [VOCAB, D], mdt, kind="ExternalInput")
    hot_d = nc.dram_tensor("hotrows", [P, hch * D], mdt, kind="ExternalInput")
    chot_d = nc.dram_tensor("chot", [P, NT * hch * P], mdt,
                            kind="ExternalInput")
    pos_d = nc.dram_tensor("pos", [P, D], mdt, kind="ExternalInput")
    idx_d = nc.dram_tensor("idxs", [P, total_chunks * 8], mybir.dt.int16,
                           kind="ExternalInput")
    w_d = nc.dram_tensor("wmat", [P, total_entries * P], mdt,
                         kind="ExternalInput")
    out_d = nc.dram_tensor("out", [R, D], mdt, kind="ExternalOutput")

    # build-time schedule: same policy as the cost sim in _prepare
    q7 = Q7_START_NS
    ready = []
    for call in calls:
        q7 += Q7_CALL_NS + call["num_idxs"] * Q7_IDX_NS
        ready.append(q7 + READY_NS)
    hot_dur = hch * 2 * MM_NS
    sched = []               # ("h", t) | ("c", ci)
    tile_order = [t for g in ginfo for t in g["tiles"]]
    hot_done = {t: False for t in tile_order}
    ndone = 0
    ndrained = 0
    pe = PE_START_NS
    nexth = 0
    for ci, call in enumerate(calls):
        assert call["nch"] <= MAXCH
        ctiles = sorted({t for _, t in call["entries"]})
        while pe < ready[ci] and nexth < NT:
            t = tile_order[nexth]
            if hot_done[t]:
                nexth += 1
                continue
            if ndone - ndrained >= 4:
                break
            hot_done[t] = True
            ndone += 1
            sched.append(("h", t))
            pe += hot_dur
        for t in ctiles:
            if not hot_done[t]:
                hot_done[t] = True
                ndone += 1
                sched.append(("h", t))
                pe += hot_dur
        sched.append(("c", ci))
        pe = max(pe, ready[ci]) + len(call["entries"]) * 2 * MM_NS
        ndrained += len(call["last"])
    for t in tile_order:
        if not hot_done[t]:
            sched.append(("h", t))

    # num_idxs registers hoisted before the tile scope so the MOVEs run
    # during the preamble, not on the critical Pool-sequencer path
    nregs = {}
    for call in calls:
        n = call["num_idxs"]
        if n not in nregs:
            nregs[n] = nc.gpsimd.to_reg(n)

    with tile.TileContext(nc) as tc:
        with (
            tc.tile_pool(name="const", bufs=1) as cpool,
            tc.tile_pool(name="gather", bufs=8) as gpool,
            tc.tile_pool(name="wpool", bufs=4) as wpool,
            tc.tile_pool(name="osb", bufs=6) as opool,
            tc.tile_pool(name="psum", bufs=4, space=bass.MemorySpace.PSUM) as ppool,
        ):
            # keep the early HBM burst minimal: a saturated HBM window at
            # kernel start starves the Pool sequencer and delays the first
            # gather by ~10us. Load idx + hot rows + the first few chot
            # slices now; pace the rest between gather calls.
            idx_sb = cpool.tile([P, total_chunks * 8], mybir.dt.int16)
            cut = calls[0]["nch"] * 8
            nc.sync.dma_start(idx_sb[:, :cut], idx_d[:, :cut])
            nc.sync.dma_start(idx_sb[:, cut:], idx_d[:, cut:])
            hot_sb = cpool.tile([P, hch * D], mdt)
            nc.sync.dma_start(hot_sb[:], hot_d[:])
            chot_sb = cpool.tile([P, NT * hch * P], mdt)

            def load_chot(t):
                lo, hi = t * hch * P, (t + 1) * hch * P
                nc.sync.dma_start(chot_sb[:, lo:hi], chot_d[:, lo:hi])

            chot_loaded = set()
            for t in tile_order[:4]:
                load_chot(t)
                chot_loaded.add(t)
            pos_sb = cpool.tile([P, D], mdt)

            # PE p-state warmup on a zeroed dummy block
            wz = cpool.tile([P, P + 512], mdt)
            nc.scalar.memzero(wz[:])
            if NWARM:
                pwarm = ppool.tile([P, 512], f32, tag="ps")
                for i in range(NWARM):
                    nc.tensor.matmul(
                        pwarm[:],
                        lhsT=wz[:, :P],
                        rhs=wz[:, P:P + 512],
                        start=(i == 0),
                        stop=(i == NWARM - 1),
                    )

            def prefetch_chot(n):
                for t in tile_order:
                    if n <= 0:
                        break
                    if t not in chot_loaded:
                        load_chot(t)
                        chot_loaded.add(t)
                        n -= 1

            psums = {}
            ncalls_done = 0
            for kind, item in sched:
                if kind == "h":
                    t = item
                    if t not in chot_loaded:
                        load_chot(t)
                        chot_loaded.add(t)
                    psums[t] = ppool.tile([P, D], f32, tag="ps", name="ps")
                    for k in range(hch):
                        base = (t * hch + k) * P
                        for h in range(0, D, 512):
                            nc.tensor.matmul(
                                psums[t][:, h:h + 512],
                                lhsT=chot_sb[:, base:base + P],
                                rhs=hot_sb[:, k * D + h:k * D + h + 512],
                                start=(k == 0),
                                stop=False,
                            )
                else:
                    call = calls[item]
                    nch = call["nch"]
                    b0 = call["idx_base"]
                    prefetch_chot(3)
                    if ncalls_done == 1:
                        nc.sync.dma_start(pos_sb[:], pos_d[:])
                    ncalls_done += 1
                    gt = gpool.tile([P, MAXCH, D], mdt, tag="gt")
                    nc.gpsimd.dma_gather(
                        gt[:, :nch, :],
                        table_d[:],
                        idx_sb[:, b0 * 8:(b0 + nch) * 8],
                        num_idxs=call["num_idxs"],
                        num_idxs_reg=nregs[call["num_idxs"]],
                        elem_size=D,
                    )
                    ne = len(call["entries"])
                    wt = wpool.tile([P, max_entries * P], mdt, tag="wt")
                    eb = call["entry_base"]
                    nc.sync.dma_start(
                        wt[:, :ne * P], w_d[:, eb * P:(eb + ne) * P]
                    )
                    for e, (j, t) in enumerate(call["entries"]):
                        last = call["last"].get(t, -1) == e
                        for h in range(0, D, 512):
                            nc.tensor.matmul(
                                psums[t][:, h:h + 512],
                                lhsT=wt[:, e * P:(e + 1) * P],
                                rhs=gt[:, j, h:h + 512],
                                start=False,
                                stop=last,
                            )
                        if last:
                            o1 = opool.tile([P, D], mdt, tag="o1")
                            nc.scalar.copy(o1[:], psums[t][:])
                            o2 = opool.tile([P, D], mdt, tag="o2")
                            nc.vector.tensor_tensor(
                                o2[:], o1[:], pos_sb[:],
                                op=mybir.AluOpType.add,
                            )
                            nc.sync.dma_start(
                                out_d[t * P:(t + 1) * P, :], o2[:]
                            )
    nc.compile()
    return nc


def run(inputs, trace=False, tmpdir=None):
    """Returns (out [B,S,D] f32, BassKernelResults)."""
    plan, idx_maps, w_maps, chot_maps, hot_maps = _prepare(
        inputs["phon_tokens"], inputs["group_len_raw"]
    )
    wdt = ml_dtypes.bfloat16
    hch = plan["hch"]
    table_np = np.ascontiguousarray(
        np.asarray(inputs["phon_emb_table"]).astype(wdt)
    )
    pos_np = np.ascontiguousarray(
        np.asarray(inputs["pos_emb_table"]).astype(wdt)
    )

    nc = _build_nc(plan)
    in_maps = []
    for c in range(NCORES):
        hot_rows = table_np[hot_maps[c]]          # [hch*128, D]
        hot_np = np.ascontiguousarray(
            hot_rows.reshape(hch, P, D).transpose(1, 0, 2).reshape(P, hch * D)
        )
        in_maps.append({
            "table": table_np, "hotrows": hot_np, "chot": chot_maps[c],
            "pos": pos_np, "idxs": idx_maps[c], "wmat": w_maps[c],
        })
    res = run_bass_kernel_spmd(
        nc, in_maps, core_ids=list(range(NCORES)), trace=trace, tmpdir=tmpdir
    )
    out = np.empty((B, S, D), np.float32)
    for c in range(NCORES):
        out[c * BPC:(c + 1) * BPC] = (
            res.results[c]["out"].astype(np.float32).reshape(BPC, S, D)
        )
    return out, res


def kernel(**inputs) -> np.ndarray:
    out, _ = run(inputs, trace=False)
    return out
